# revision 34
# baseline (speedup 1.0000x reference)
"""GQA attention kernel for 8 Trainium2 NeuronCores.

Sharding: core c -> (b = c // 4, kv-group gk = c % 4).
Each core computes, for its batch b and its kv head gk (which owns the 4
contiguous q-heads gk*4..gk*4+3):
    q/k/v projections, attention, and a partial out-projection
    out_partial[b] = o_heads @ Wo[:, gk*512:(gk+1)*512].T
Host sums the 4 partials per batch (bf16 partials, f32 accumulate).

All matmuls in bf16 (fp32 PSUM accumulation). Softmax without max
subtraction (scores are bounded ~|4.5| at this problem's weight scale);
row sums come free from a ones-column appended to V; normalization is
applied to the 128-wide per-head output ahead of the out projection.

Layout (per core), everything E/K-major for the PE:
  xT  [E, N]   = x[b].T          kT [128d, N]    scoresT [s, n] chunks
  wq  [E, 512] = Wq rows.T       qT [128, 4g, N]
  wk  [E, 128] = Wk rows.T       v  [128, 16st, 130] (col 128 = ones)
  wv  [E, 128]                   oT [128, 4g, N]
  wo  [512, E] = Wo cols.T       out [N, E] bf16 partial

Schedule notes (from NTFF profiling):
 - only sync/scalar drive fast (~140GB/s) HW DMA queues; gpsimd's
   software queue is ~35GB/s and its slow descriptors hog the shared
   credit pool, so it carries just one late x chunk. All input DMAs are
   issued in consumption order so the first k matmul starts ~10us in;
 - dummy matmuls on the identity ramp the PE out of its low p-state
   while the first x chunk is in flight;
 - all 16 v s-tile PSUM accumulators run concurrently (4 per bank;
   start=True only on each bank's first slice since it clears the whole
   bank's has_written bits), so k+v trail the x stream without stalls;
 - kT/v copies ride the ACT engine, the DVE handles only q casts;
 - o-groups trail their chunk's exps by >= 4 score-pairs, their PE
   transposes are deferred one slot behind the DVE normalization, and
   the out-projection runs in single-bank P2 accumulators spread one
   half-tile per slot, so neither ACT, DVE, nor PSUM rings gate the PE.
"""

import sys

sys.path.insert(0, "/opt/trn_rl_repo")

import numpy as np
import ml_dtypes

import concourse.bass as bass
import concourse.mybir as mybir
import concourse.tile as tile
from concourse import bacc
from concourse.bass_utils import run_bass_kernel_spmd
from concourse.masks import make_identity

BF16 = mybir.dt.bfloat16
F32 = mybir.dt.float32
bf16 = ml_dtypes.bfloat16

B, N, E = 2, 2048, 2048
H, D, G = 16, 128, 4
HKV = H // G
JL = G * D                     # 512 local q-head dims per core
ET = E // 128                  # 16
NT = N // 128                  # 16
CH = N // 512                  # 4
SCALE = 1.0 / float(np.sqrt(D))

_cached = {}


def _build(iters=1):
    nc = bacc.Bacc("TRN2", target_bir_lowering=False, debug=False, num_devices=8)

    xT = nc.dram_tensor("xT", [E, N], BF16, kind="ExternalInput")
    wq = nc.dram_tensor("wq", [E, JL], BF16, kind="ExternalInput")
    wk = nc.dram_tensor("wk", [E, D], BF16, kind="ExternalInput")
    wv = nc.dram_tensor("wv", [E, D], BF16, kind="ExternalInput")
    wo = nc.dram_tensor("wo", [JL, E], BF16, kind="ExternalInput")
    out = nc.dram_tensor("out", [N, E], BF16, kind="ExternalOutput")

    with tile.TileContext(nc) as tc:
        with (
            tc.tile_pool(name="const", bufs=1) as cpool,
            tc.tile_pool(name="xp", bufs=1) as xpool,
            tc.tile_pool(name="wp", bufs=1) as wpool,
            tc.tile_pool(name="kvp", bufs=1) as kvpool,
            tc.tile_pool(name="qp", bufs=1) as qpool,
            tc.tile_pool(name="pp", bufs=3) as ppool,
            tc.tile_pool(name="op", bufs=4) as opool,
            tc.tile_pool(name="otp", bufs=1) as otpool,
            tc.tile_pool(name="outp", bufs=4) as outpool,
            tc.tile_pool(name="ps1", bufs=4, space="PSUM") as P1,
        ):
            ident = cpool.tile([128, 128], BF16, tag="ident")
            make_identity(nc, ident[:])

            for _ in range(iters):
                _emit_iter(nc, tc, ident, xpool, wpool, kvpool, qpool, ppool,
                           opool, otpool, outpool, P1,
                           xT, wq, wk, wv, wo, out)

    nc.compile()
    return nc


def _emit_iter(nc, tc, ident, xpool, wpool, kvpool, qpool, ppool, opool,
               otpool, outpool, P1, xT, wq, wk, wv, wo, out):
    x_sb = xpool.tile([128, ET, N], BF16, tag="x")
    wq_sb = wpool.tile([128, ET, JL], BF16, tag="wq")
    wk_sb = wpool.tile([128, ET, D], BF16, tag="wk")
    wv_sb = wpool.tile([128, ET, D], BF16, tag="wv")
    wo_sb = wpool.tile([128, G, E], BF16, tag="wo")
    kT_sb = kvpool.tile([128, N], BF16, tag="kT")
    v_sb = kvpool.tile([128, NT, 130], BF16, tag="v")
    qT_sb = qpool.tile([128, G, N], BF16, tag="qT")
    oT_sb = otpool.tile([128, G, N], BF16, tag="oT")

    # --- input DMAs, in consumption order ---
    # Only sync and scalar drive fast hardware DMA queues (~140GB/s
    # each). The gpsimd queue is software-driven (~35GB/s) AND its slow
    # descriptors hog the shared DMA credit pool, freezing the fast
    # queues — so it gets nothing. Everything rides the two fast queues
    # in consumption order: x0/wk first, wq/wo behind x (needed late).
    xr = xT.rearrange("(a p) n -> p a n", p=128)
    wkr = wk.rearrange("(a p) d -> p a d", p=128)
    wvr = wv.rearrange("(a p) d -> p a d", p=128)
    wqr = wq.rearrange("(a p) j -> p a j", p=128)
    nc.scalar.dma_start(x_sb[:, 0, 0:1024], xr[:, 0, 0:1024])
    nc.sync.dma_start(wk_sb[:, 0:8, :], wkr[:, 0:8, :])
    nc.scalar.dma_start(x_sb[:, 0, 1024:2048], xr[:, 0, 1024:2048])
    nc.sync.dma_start(wk_sb[:, 8:16, :], wkr[:, 8:16, :])
    nc.scalar.dma_start(wv_sb[:, 0:8, :], wvr[:, 0:8, :])
    nc.scalar.dma_start(x_sb[:, 1, :], xr[:, 1, :])
    nc.sync.dma_start(wv_sb[:, 8:16, :], wvr[:, 8:16, :])
    # x15 rides the slow gpsimd queue: issued up front it finishes ~37us,
    # right when the k/v et-loop reaches it, and a single credit can't
    # clog the shared descriptor pool.
    nc.gpsimd.dma_start(x_sb[:, 15, :], xr[:, 15, :])
    for et in range(2, ET - 1):
        eng = nc.sync if et % 2 == 0 else nc.scalar
        eng.dma_start(x_sb[:, et, :], xr[:, et, :])
    for g in range(G):
        nc.sync.dma_start(wq_sb[:, :, g * 128:(g + 1) * 128],
                          wqr[:, :, g * 128:(g + 1) * 128])
    for jt in range(G):
        nc.scalar.dma_start(wo_sb[:, jt, :], wo[jt * 128:(jt + 1) * 128, :])

    nc.vector.memset(v_sb[:, :, 128:129], 1.0)

    # --- phase 1 (own PSUM scope: k 4 banks + v 4 banks) ---
    # kT: 4 chunk accumulators (2 double-bank tiles) and ALL 16 v s-tile
    # accumulators (4 per bank as [128,128] f32 slices) run concurrently,
    # so the whole k+v projection (28us of PE) trails the x DMA stream
    # (~30us) with no starvation window.
    if True:
        # PE warmup: dummy matmuls on the identity while the first DMAs
        # are in flight, ramping the PE out of its low p-state.
        for i in range(24):
            wmt = P1.tile([128, 512], F32, tag="mm", name=f"warm{i}")
            nc.tensor.matmul(wmt[:, 0:128], ident[:], ident[:],
                             start=True, stop=True)

        kp = [P1.tile([128, 1024], F32, tag="mm", name=f"kp{_i}")
              for _i in range(2)]
        kps = [kp[_i // 2][:, (_i % 2) * 512:(_i % 2 + 1) * 512]
               for _i in range(CH)]
        vacc = [P1.tile([128, 1024], F32, tag="mm", name=f"vacc{_i}")
                for _i in range(2)]
        vps = [vacc[_s // 8][:, (_s % 8) * 128:(_s % 8 + 1) * 128]
               for _s in range(NT)]
        # The v et-steps lag k's by 4, so early PE consumption (~0.85us/et
        # k-only) matches the slower early x arrivals, then k+v (~2.9us/et)
        # trails the steady stream with no starvation.
        def v_step(et):
            for st in range(NT):
                # start=True clears has_written for the WHOLE bank, so only
                # the first slice of each 4-slice bank may use it; the other
                # slices' first writes overwrite-and-set on cleared bits,
                # which is exactly accumulation-start semantics.
                nc.tensor.matmul(
                    vps[st], x_sb[:, et, st * 128:(st + 1) * 128],
                    wv_sb[:, et, :],
                    start=(et == 0 and st % 4 == 0), stop=(et == ET - 1),
                )

        for et in range(ET):
            for sc in range(CH):
                nc.tensor.matmul(
                    kps[sc], wk_sb[:, et, :], x_sb[:, et, sc * 512:(sc + 1) * 512],
                    start=(et == 0), stop=(et == ET - 1),
                )
            if et >= 4:
                v_step(et - 4)
        for et in range(ET - 4, ET):
            v_step(et)
        # kT/v copies go on the (otherwise idle) ACT engine so the DVE
        # queue holds nothing but the q-pair casts (their P1 ring readers).
        kv_copies = [lambda sc=sc: nc.scalar.copy(
                         kT_sb[:, sc * 512:(sc + 1) * 512], kps[sc])
                     for sc in range(CH)]
        kv_copies += [lambda st=st: nc.scalar.copy(v_sb[:, st, 0:128], vps[st])
                      for st in range(NT)]

        def emit_q_pair(q0, q1):
            ps = P1.tile([128, 1024], F32, tag="mm")
            for half, (g, ncg) in enumerate((q0, q1)):
                sl = ps[:, half * 512:(half + 1) * 512]
                for et in range(ET):
                    nc.tensor.matmul(
                        sl, wq_sb[:, et, g * 128:(g + 1) * 128],
                        x_sb[:, et, ncg * 512:(ncg + 1) * 512],
                        start=(et == 0), stop=(et == ET - 1),
                    )
                nc.vector.tensor_copy(qT_sb[:, g, ncg * 512:(ncg + 1) * 512], sl)

        # kp0's copies must precede qp0 (P1 ring), kp1's must precede qp1;
        # the v copies (needed only by attention) fill the remaining gaps.
        qlist = [(g, ncg) for g in range(G) for ncg in range(CH)]
        for i in range(8):
            if i < 2:
                kv_copies.pop(0)()
                kv_copies.pop(0)()
            emit_q_pair(qlist[2 * i], qlist[2 * i + 1])
            for _ in range(3):
                if kv_copies:
                    kv_copies.pop(0)()

    # --- phase 2 + 3, pipelined per chunk of 512 n-columns ---
    # Scores for two s-tiles share one double-bank psum tile so a single
    # (wider, cheaper per element) Exp covers both. o-groups trail their
    # chunk by >= 2 score-pairs so the PE never catches the ACT engine;
    # one out-projection n-tile is emitted per sub-slot once a column's
    # four heads are done.
    if True:
        def emit_score_pair(g, c, p_t, sp):
            ps = P1.tile([128, 1024], F32, tag="mm")
            for half in range(2):
                st = 2 * sp + half
                nc.tensor.matmul(
                    ps[:, half * 512:(half + 1) * 512],
                    kT_sb[:, st * 128:(st + 1) * 128],
                    qT_sb[:, g, c * 512:(c + 1) * 512],
                    start=True, stop=True,
                )
            nc.scalar.activation(
                p_t[:, 2 * sp * 512:(2 * sp + 2) * 512], ps[:],
                mybir.ActivationFunctionType.Exp, scale=SCALE,
            )

        def emit_o_group_av(g, c, p_t, t):
            """AV matmuls + normalization (DVE); returns the transpose step,
            which the caller defers a slot so the PE never waits on the DVE."""
            pso = P1.tile([128, 130], F32, tag="mm")
            for st in range(NT):
                nc.tensor.matmul(
                    pso[:, 0:129], p_t[:, st * 512 + t * 128: st * 512 + (t + 1) * 128],
                    v_sb[:, st, 0:129],
                    start=(st == 0), stop=(st == NT - 1),
                )
            rc = opool.tile([128, 1], F32, tag="recip")
            nc.vector.reciprocal(rc[:], pso[:, 128:129])
            o_n = opool.tile([128, 128], BF16, tag="o_n")
            nc.vector.tensor_scalar_mul(o_n[:], pso[:, 0:128], rc[:])

            def transpose_step():
                pst = P1.tile([128, 128], BF16, tag="mm")
                nc.tensor.transpose(pst[:], o_n[:], ident[:])
                nc.vector.tensor_copy(
                    oT_sb[:, g, c * 512 + t * 128: c * 512 + (t + 1) * 128], pst[:],
                )
            return transpose_step

        def emit_out_half(nt, half):
            ps = P1.tile([128, 1024], F32, tag="mm")
            pe2 = [ps[:, 0:512], ps[:, 512:1024]]
            for e2 in range(2):
                ec = half * 2 + e2
                for g in range(G):
                    nc.tensor.matmul(
                        pe2[e2], oT_sb[:, g, nt * 128:(nt + 1) * 128],
                        wo_sb[:, g, ec * 512:(ec + 1) * 512],
                        start=(g == 0), stop=(g == G - 1),
                    )
            stage = outpool.tile([128, 1024], BF16, tag="out")
            # In the tail column the second cast runs on the (then idle)
            # ACT engine, splitting the drain across both copy engines.
            nc.vector.tensor_copy(stage[:, 0:512], pe2[0])
            if nt >= 12:
                nc.scalar.copy(stage[:, 512:1024], pe2[1])
            else:
                nc.vector.tensor_copy(stage[:, 512:1024], pe2[1])
            nc.sync.dma_start(
                out[nt * 128:(nt + 1) * 128, half * 1024:(half + 1) * 1024],
                stage[:],
            )

        # pending o-group / out-half work queue: one entry per "slot"
        # (after each sub's score pairs), two when backed up. o-groups
        # never pop at a chunk's first slot (guarantees >= 4 score-pairs
        # of exp lead); their transposes are deferred one slot (returned
        # as follow-ups) so the PE never waits on the DVE normalization.
        chunks = [(c, g) for c in range(CH) for g in range(G)]
        pending = []
        follow = []

        def pop_one():
            kind, fn = pending.pop(0)
            r = fn()
            if r is not None:
                follow.append(r)

        def pop_slot(sub):
            while follow:
                follow.pop(0)()
            if pending and not (sub == 0 and pending[0][0] == "og"):
                pop_one()
            if len(pending) > 5:
                pop_one()

        for i, (c, g) in enumerate(chunks):
            p_t = ppool.tile([128, NT * 512], BF16, tag="p", name=f"p{i}")
            for sub in range(4):
                emit_score_pair(g, c, p_t, 2 * sub)
                emit_score_pair(g, c, p_t, 2 * sub + 1)
                pop_slot(sub)
            # queue this chunk's o-groups (consumed over the next chunks);
            # when the column's 4 heads are done, interleave the column's
            # out-projection halves behind the o-groups that produce them.
            og = [("og", lambda g=g, c=c, p_t=p_t, t=t: emit_o_group_av(g, c, p_t, t))
                  for t in range(4)]
            if g == G - 1:
                oh = [("oh", lambda nt=4 * c + t, h=h: emit_out_half(nt, h))
                      for t in range(4) for h in range(2)]
                pending.extend([og[0], og[1], og[2], oh[0], og[3], oh[1],
                                oh[2], oh[3], oh[4], oh[5], oh[6], oh[7]])
            else:
                pending.extend(og)
        while pending:
            pop_slot(1)
        while follow:
            follow.pop(0)()


def get_nc(iters=1):
    key = ("nc", iters)
    if key not in _cached:
        _cached[key] = _build(iters)
    return _cached[key]


def make_in_maps(x, Wq, Wk, Wv, Wo):
    """Per-core host-side sharding. Core c -> (b=c//4, gk=c%4)."""
    in_maps = []
    xT = [np.ascontiguousarray(x[b].T).astype(bf16) for b in range(B)]
    wq_s = [np.ascontiguousarray(Wq[gk * JL:(gk + 1) * JL, :].T).astype(bf16)
            for gk in range(HKV)]
    wk_s = [np.ascontiguousarray(Wk[gk * D:(gk + 1) * D, :].T).astype(bf16)
            for gk in range(HKV)]
    wv_s = [np.ascontiguousarray(Wv[gk * D:(gk + 1) * D, :].T).astype(bf16)
            for gk in range(HKV)]
    wo_s = [np.ascontiguousarray(Wo[:, gk * JL:(gk + 1) * JL].T).astype(bf16)
            for gk in range(HKV)]
    for c in range(8):
        b, gk = c // 4, c % 4
        in_maps.append({
            "xT": xT[b], "wq": wq_s[gk], "wk": wk_s[gk],
            "wv": wv_s[gk], "wo": wo_s[gk],
        })
    return in_maps


def kernel(x, Wq, Wk, Wv, Wo):
    nc = get_nc()
    in_maps = make_in_maps(x, Wq, Wk, Wv, Wo)
    res = run_bass_kernel_spmd(nc, in_maps, core_ids=list(range(8)))
    out = np.empty((B, N, E), np.float32)
    for b in range(B):
        acc = res.results[b * 4]["out"].astype(np.float32)
        for gk in range(1, HKV):
            acc = acc + res.results[b * 4 + gk]["out"].astype(np.float32)
        out[b] = acc
    return out


# revision 35
# speedup vs baseline: 1.1837x; 1.1837x over previous
"""GQA attention kernel for 8 Trainium2 NeuronCores.

Sharding: core c -> (b = c // 4, kv-group gk = c % 4).
Each core computes, for its batch b and its kv head gk (which owns the 4
contiguous q-heads gk*4..gk*4+3):
    q/k/v projections, attention, and a partial out-projection
    out_partial[b] = o_heads @ Wo[:, gk*512:(gk+1)*512].T
Host sums the 4 partials per batch (bf16 partials, f32 accumulate).

All matmuls in bf16 (fp32 PSUM accumulation). Softmax without max
subtraction (scores are bounded ~|4.5| at this problem's weight scale);
row sums come free from a ones-column appended to V; normalization is
applied to the 128-wide per-head output ahead of the out projection.

Layout (per core), everything E/K-major for the PE:
  xT  [E, N]   = x[b].T          kT [128d, N]    scoresT [s, n] chunks
  wq  [E, 512] = Wq rows.T       qT [128, 4g, N]
  wk  [E, 128] = Wk rows.T       v  [128, 16st, 130] (col 128 = ones)
  wv  [E, 128]                   oT [128, 4g, N]
  wo  [512, E] = Wo cols.T       out [N, E] bf16 partial

Schedule notes (from NTFF profiling):
 - only sync/scalar drive fast (~140GB/s) HW DMA queues; gpsimd's
   software queue is ~35GB/s and its slow descriptors hog the shared
   credit pool, so it carries just one late x chunk. All input DMAs are
   issued in consumption order so the first k matmul starts ~10us in;
 - dummy matmuls on the identity ramp the PE out of its low p-state
   while the first x chunk is in flight;
 - one unified PSUM ring: a single [128,1024]-slot tag, bufs=4 (all 8
   banks). Phase 1 holds exactly 4 slots (k chunks x2, v x2 with eight
   [128,128] accumulator slices each; start=True only on each bank's
   first slice since it clears the whole bank's has_written bits), so
   k+v trail the x stream concurrently; attention cycles the same ring
   for score pairs, AV groups, transposes, and out-projection halves;
 - kT/v copies ride the ACT engine, the DVE handles only q casts;
 - o-groups trail their chunk's exps by >= 4 score-pairs, their PE
   transposes are deferred one slot behind the DVE normalization, and
   the out-projection is spread one half-tile per slot, so neither
   ACT, DVE, nor the PSUM ring gates the PE.
"""

import sys

sys.path.insert(0, "/opt/trn_rl_repo")

import numpy as np
import ml_dtypes

import concourse.bass as bass
import concourse.mybir as mybir
import concourse.tile as tile
from concourse import bacc
from concourse.bass_utils import run_bass_kernel_spmd
from concourse.masks import make_identity

BF16 = mybir.dt.bfloat16
F32 = mybir.dt.float32
bf16 = ml_dtypes.bfloat16

B, N, E = 2, 2048, 2048
H, D, G = 16, 128, 4
HKV = H // G
JL = G * D                     # 512 local q-head dims per core
ET = E // 128                  # 16
NT = N // 128                  # 16
CH = N // 512                  # 4
SCALE = 1.0 / float(np.sqrt(D))

_cached = {}


def _build(iters=1):
    nc = bacc.Bacc("TRN2", target_bir_lowering=False, debug=False, num_devices=8)

    xT = nc.dram_tensor("xT", [E, N], BF16, kind="ExternalInput")
    wq = nc.dram_tensor("wq", [E, JL], BF16, kind="ExternalInput")
    wk = nc.dram_tensor("wk", [E, D], BF16, kind="ExternalInput")
    wv = nc.dram_tensor("wv", [E, D], BF16, kind="ExternalInput")
    wo = nc.dram_tensor("wo", [JL, E], BF16, kind="ExternalInput")
    out = nc.dram_tensor("out", [N, E], BF16, kind="ExternalOutput")

    with tile.TileContext(nc) as tc:
        with (
            tc.tile_pool(name="const", bufs=1) as cpool,
            tc.tile_pool(name="xp", bufs=1) as xpool,
            tc.tile_pool(name="wp", bufs=1) as wpool,
            tc.tile_pool(name="kvp", bufs=1) as kvpool,
            tc.tile_pool(name="qp", bufs=1) as qpool,
            tc.tile_pool(name="pp", bufs=3) as ppool,
            tc.tile_pool(name="op", bufs=4) as opool,
            tc.tile_pool(name="otp", bufs=1) as otpool,
            tc.tile_pool(name="outp", bufs=4) as outpool,
            tc.tile_pool(name="ps1", bufs=4, space="PSUM") as P1,
        ):
            ident = cpool.tile([128, 128], BF16, tag="ident")
            make_identity(nc, ident[:])

            for _ in range(iters):
                _emit_iter(nc, tc, ident, xpool, wpool, kvpool, qpool, ppool,
                           opool, otpool, outpool, P1,
                           xT, wq, wk, wv, wo, out)

    nc.compile()
    return nc


def _emit_iter(nc, tc, ident, xpool, wpool, kvpool, qpool, ppool, opool,
               otpool, outpool, P1, xT, wq, wk, wv, wo, out):
    x_sb = xpool.tile([128, ET, N], BF16, tag="x")
    wq_sb = wpool.tile([128, ET, JL], BF16, tag="wq")
    wk_sb = wpool.tile([128, ET, D], BF16, tag="wk")
    wv_sb = wpool.tile([128, ET, D], BF16, tag="wv")
    wo_sb = wpool.tile([128, G, E], BF16, tag="wo")
    kT_sb = kvpool.tile([128, N], BF16, tag="kT")
    v_sb = kvpool.tile([128, NT, 130], BF16, tag="v")
    qT_sb = qpool.tile([128, G, N], BF16, tag="qT")
    oT_sb = otpool.tile([128, G, N], BF16, tag="oT")

    # --- input DMAs, in consumption order ---
    # Only sync and scalar drive fast hardware DMA queues (~140GB/s
    # each). The gpsimd queue is software-driven (~35GB/s) AND its slow
    # descriptors hog the shared DMA credit pool, freezing the fast
    # queues — so it gets nothing. Everything rides the two fast queues
    # in consumption order: x0/wk first, wq/wo behind x (needed late).
    xr = xT.rearrange("(a p) n -> p a n", p=128)
    wkr = wk.rearrange("(a p) d -> p a d", p=128)
    wvr = wv.rearrange("(a p) d -> p a d", p=128)
    wqr = wq.rearrange("(a p) j -> p a j", p=128)
    nc.scalar.dma_start(x_sb[:, 0, 0:1024], xr[:, 0, 0:1024])
    nc.sync.dma_start(wk_sb[:, 0:8, :], wkr[:, 0:8, :])
    nc.scalar.dma_start(x_sb[:, 0, 1024:2048], xr[:, 0, 1024:2048])
    nc.sync.dma_start(wk_sb[:, 8:16, :], wkr[:, 8:16, :])
    nc.scalar.dma_start(wv_sb[:, 0:8, :], wvr[:, 0:8, :])
    nc.scalar.dma_start(x_sb[:, 1, :], xr[:, 1, :])
    nc.sync.dma_start(wv_sb[:, 8:16, :], wvr[:, 8:16, :])
    # x15 rides the slow gpsimd queue: issued up front it finishes ~37us,
    # right when the k/v et-loop reaches it, and a single credit can't
    # clog the shared descriptor pool.
    nc.gpsimd.dma_start(x_sb[:, 15, :], xr[:, 15, :])
    for et in range(2, ET - 1):
        eng = nc.sync if et % 2 == 0 else nc.scalar
        eng.dma_start(x_sb[:, et, :], xr[:, et, :])
    for g in range(G):
        nc.sync.dma_start(wq_sb[:, :, g * 128:(g + 1) * 128],
                          wqr[:, :, g * 128:(g + 1) * 128])
    for jt in range(G):
        nc.scalar.dma_start(wo_sb[:, jt, :], wo[jt * 128:(jt + 1) * 128, :])

    nc.vector.memset(v_sb[:, :, 128:129], 1.0)

    # --- phase 1 (own PSUM scope: k 4 banks + v 4 banks) ---
    # kT: 4 chunk accumulators (2 double-bank tiles) and ALL 16 v s-tile
    # accumulators (4 per bank as [128,128] f32 slices) run concurrently,
    # so the whole k+v projection (28us of PE) trails the x DMA stream
    # (~30us) with no starvation window.
    if True:
        # PE warmup: dummy matmuls on the identity while the first DMAs
        # are in flight, ramping the PE out of its low p-state.
        for i in range(24):
            wmt = P1.tile([128, 512], F32, tag="mm", name=f"warm{i}")
            nc.tensor.matmul(wmt[:, 0:128], ident[:], ident[:],
                             start=True, stop=True)

        kp = [P1.tile([128, 1024], F32, tag="mm", name=f"kp{_i}")
              for _i in range(2)]
        kps = [kp[_i // 2][:, (_i % 2) * 512:(_i % 2 + 1) * 512]
               for _i in range(CH)]
        vacc = [P1.tile([128, 1024], F32, tag="mm", name=f"vacc{_i}")
                for _i in range(2)]
        vps = [vacc[_s // 8][:, (_s % 8) * 128:(_s % 8 + 1) * 128]
               for _s in range(NT)]
        # The v et-steps lag k's by 4, so early PE consumption (~0.85us/et
        # k-only) matches the slower early x arrivals, then k+v (~2.9us/et)
        # trails the steady stream with no starvation.
        def v_step(et):
            for st in range(NT):
                # start=True clears has_written for the WHOLE bank, so only
                # the first slice of each 4-slice bank may use it; the other
                # slices' first writes overwrite-and-set on cleared bits,
                # which is exactly accumulation-start semantics.
                nc.tensor.matmul(
                    vps[st], x_sb[:, et, st * 128:(st + 1) * 128],
                    wv_sb[:, et, :],
                    start=(et == 0 and st % 4 == 0), stop=(et == ET - 1),
                )

        for et in range(ET):
            for sc in range(CH):
                nc.tensor.matmul(
                    kps[sc], wk_sb[:, et, :], x_sb[:, et, sc * 512:(sc + 1) * 512],
                    start=(et == 0), stop=(et == ET - 1),
                )
            if et >= 4:
                v_step(et - 4)
        for et in range(ET - 4, ET):
            v_step(et)
        # kT/v copies go on the (otherwise idle) ACT engine so the DVE
        # queue holds nothing but the q-pair casts (their P1 ring readers).
        kv_copies = [lambda sc=sc: nc.scalar.copy(
                         kT_sb[:, sc * 512:(sc + 1) * 512], kps[sc])
                     for sc in range(CH)]
        kv_copies += [lambda st=st: nc.scalar.copy(v_sb[:, st, 0:128], vps[st])
                      for st in range(NT)]

        def emit_q_pair(q0, q1):
            ps = P1.tile([128, 1024], F32, tag="mm")
            for half, (g, ncg) in enumerate((q0, q1)):
                sl = ps[:, half * 512:(half + 1) * 512]
                for et in range(ET):
                    nc.tensor.matmul(
                        sl, wq_sb[:, et, g * 128:(g + 1) * 128],
                        x_sb[:, et, ncg * 512:(ncg + 1) * 512],
                        start=(et == 0), stop=(et == ET - 1),
                    )
                nc.vector.tensor_copy(qT_sb[:, g, ncg * 512:(ncg + 1) * 512], sl)

        # kp0's copies must precede qp0 (P1 ring), kp1's must precede qp1;
        # the v copies (needed only by attention) fill the remaining gaps.
        qlist = [(g, ncg) for g in range(G) for ncg in range(CH)]
        for i in range(8):
            if i < 2:
                kv_copies.pop(0)()
                kv_copies.pop(0)()
            emit_q_pair(qlist[2 * i], qlist[2 * i + 1])
            for _ in range(3):
                if kv_copies:
                    kv_copies.pop(0)()

    # --- phase 2 + 3, pipelined per chunk of 512 n-columns ---
    # Scores for two s-tiles share one double-bank psum tile so a single
    # (wider, cheaper per element) Exp covers both. o-groups trail their
    # chunk by >= 2 score-pairs so the PE never catches the ACT engine;
    # one out-projection n-tile is emitted per sub-slot once a column's
    # four heads are done.
    if True:
        def emit_score_pair(g, c, p_t, sp):
            ps = P1.tile([128, 1024], F32, tag="mm")
            for half in range(2):
                st = 2 * sp + half
                nc.tensor.matmul(
                    ps[:, half * 512:(half + 1) * 512],
                    kT_sb[:, st * 128:(st + 1) * 128],
                    qT_sb[:, g, c * 512:(c + 1) * 512],
                    start=True, stop=True,
                )
            nc.scalar.activation(
                p_t[:, 2 * sp * 512:(2 * sp + 2) * 512], ps[:],
                mybir.ActivationFunctionType.Exp, scale=SCALE,
            )

        def emit_o_group_av(g, c, p_t, t):
            """AV matmuls + normalization (DVE); returns the transpose step,
            which the caller defers a slot so the PE never waits on the DVE."""
            pso = P1.tile([128, 130], F32, tag="mm")
            for st in range(NT):
                nc.tensor.matmul(
                    pso[:, 0:129], p_t[:, st * 512 + t * 128: st * 512 + (t + 1) * 128],
                    v_sb[:, st, 0:129],
                    start=(st == 0), stop=(st == NT - 1),
                )
            rc = opool.tile([128, 1], F32, tag="recip")
            nc.vector.reciprocal(rc[:], pso[:, 128:129])
            o_n = opool.tile([128, 128], BF16, tag="o_n")
            nc.vector.tensor_scalar_mul(o_n[:], pso[:, 0:128], rc[:])

            def transpose_step():
                pst = P1.tile([128, 128], BF16, tag="mm")
                nc.tensor.transpose(pst[:], o_n[:], ident[:])
                nc.vector.tensor_copy(
                    oT_sb[:, g, c * 512 + t * 128: c * 512 + (t + 1) * 128], pst[:],
                )
            return transpose_step

        def emit_out_half(nt, half):
            ps = P1.tile([128, 1024], F32, tag="mm")
            pe2 = [ps[:, 0:512], ps[:, 512:1024]]
            for e2 in range(2):
                ec = half * 2 + e2
                for g in range(G):
                    nc.tensor.matmul(
                        pe2[e2], oT_sb[:, g, nt * 128:(nt + 1) * 128],
                        wo_sb[:, g, ec * 512:(ec + 1) * 512],
                        start=(g == 0), stop=(g == G - 1),
                    )
            stage = outpool.tile([128, 1024], BF16, tag="out")
            # In the tail column the second cast runs on the (then idle)
            # ACT engine, splitting the drain across both copy engines.
            nc.vector.tensor_copy(stage[:, 0:512], pe2[0])
            if nt >= 12:
                nc.scalar.copy(stage[:, 512:1024], pe2[1])
            else:
                nc.vector.tensor_copy(stage[:, 512:1024], pe2[1])
            nc.sync.dma_start(
                out[nt * 128:(nt + 1) * 128, half * 1024:(half + 1) * 1024],
                stage[:],
            )

        # pending o-group / out-half work queue: one entry per "slot"
        # (after each sub's score pairs), two when backed up. o-groups
        # never pop at a chunk's first slot (guarantees >= 4 score-pairs
        # of exp lead); their transposes are deferred one slot (returned
        # as follow-ups) so the PE never waits on the DVE normalization.
        chunks = [(c, g) for c in range(CH) for g in range(G)]
        pending = []
        follow = []

        def pop_one():
            kind, fn = pending.pop(0)
            r = fn()
            if r is not None:
                follow.append(r)

        def pop_slot(sub):
            while follow:
                follow.pop(0)()
            if pending and not (sub == 0 and pending[0][0] == "og"):
                pop_one()
            if len(pending) > 5:
                pop_one()

        for i, (c, g) in enumerate(chunks):
            p_t = ppool.tile([128, NT * 512], BF16, tag="p", name=f"p{i}")
            for sub in range(4):
                emit_score_pair(g, c, p_t, 2 * sub)
                emit_score_pair(g, c, p_t, 2 * sub + 1)
                pop_slot(sub)
            # queue this chunk's o-groups (consumed over the next chunks);
            # when the column's 4 heads are done, interleave the column's
            # out-projection halves behind the o-groups that produce them.
            og = [("og", lambda g=g, c=c, p_t=p_t, t=t: emit_o_group_av(g, c, p_t, t))
                  for t in range(4)]
            if g == G - 1:
                oh = [("oh", lambda nt=4 * c + t, h=h: emit_out_half(nt, h))
                      for t in range(4) for h in range(2)]
                pending.extend([og[0], og[1], og[2], oh[0], og[3], oh[1],
                                oh[2], oh[3], oh[4], oh[5], oh[6], oh[7]])
            else:
                pending.extend(og)
        while pending:
            pop_slot(1)
        while follow:
            follow.pop(0)()


def get_nc(iters=1):
    key = ("nc", iters)
    if key not in _cached:
        _cached[key] = _build(iters)
    return _cached[key]


def make_in_maps(x, Wq, Wk, Wv, Wo):
    """Per-core host-side sharding. Core c -> (b=c//4, gk=c%4)."""
    in_maps = []
    xT = [np.ascontiguousarray(x[b].T).astype(bf16) for b in range(B)]
    wq_s = [np.ascontiguousarray(Wq[gk * JL:(gk + 1) * JL, :].T).astype(bf16)
            for gk in range(HKV)]
    wk_s = [np.ascontiguousarray(Wk[gk * D:(gk + 1) * D, :].T).astype(bf16)
            for gk in range(HKV)]
    wv_s = [np.ascontiguousarray(Wv[gk * D:(gk + 1) * D, :].T).astype(bf16)
            for gk in range(HKV)]
    wo_s = [np.ascontiguousarray(Wo[:, gk * JL:(gk + 1) * JL].T).astype(bf16)
            for gk in range(HKV)]
    for c in range(8):
        b, gk = c // 4, c % 4
        in_maps.append({
            "xT": xT[b], "wq": wq_s[gk], "wk": wk_s[gk],
            "wv": wv_s[gk], "wo": wo_s[gk],
        })
    return in_maps


def kernel(x, Wq, Wk, Wv, Wo):
    nc = get_nc()
    in_maps = make_in_maps(x, Wq, Wk, Wv, Wo)
    res = run_bass_kernel_spmd(nc, in_maps, core_ids=list(range(8)))
    out = np.empty((B, N, E), np.float32)
    for b in range(B):
        acc = res.results[b * 4]["out"].astype(np.float32)
        for gk in range(1, HKV):
            acc = acc + res.results[b * 4 + gk]["out"].astype(np.float32)
        out[b] = acc
    return out


# revision 36
# speedup vs baseline: 1.2102x; 1.0224x over previous
"""GQA attention kernel for 8 Trainium2 NeuronCores.

Sharding: core c -> (b = c // 4, kv-group gk = c % 4).
Each core computes, for its batch b and its kv head gk (which owns the 4
contiguous q-heads gk*4..gk*4+3):
    q/k/v projections, attention, and a partial out-projection
    out_partial[b] = o_heads @ Wo[:, gk*512:(gk+1)*512].T
Host sums the 4 partials per batch (bf16 partials, f32 accumulate).

All matmuls in bf16 (fp32 PSUM accumulation). Softmax without max
subtraction (scores are bounded ~|4.5| at this problem's weight scale);
row sums come free from a ones-column appended to V; normalization is
applied to the 128-wide per-head output ahead of the out projection.

Layout (per core), everything E/K-major for the PE:
  xT  [E, N]   = x[b].T          kT [128d, N]    scoresT [s, n] chunks
  wq  [E, 512] = Wq rows.T       qT [128, 4g, N]
  wk  [E, 128] = Wk rows.T       v  [128, 16st, 130] (col 128 = ones)
  wv  [E, 128]                   oT [128, 4g, N]
  wo  [512, E] = Wo cols.T       out [N, E] bf16 partial

Schedule notes (from NTFF profiling):
 - only sync/scalar drive fast (~140GB/s) HW DMA queues; gpsimd's
   software queue is ~35GB/s and its slow descriptors hog the shared
   credit pool, so it carries just one late x chunk. All input DMAs are
   issued in consumption order so the first k matmul starts ~10us in;
 - dummy matmuls on the identity ramp the PE out of its low p-state
   while the first x chunk is in flight;
 - one unified PSUM ring: a single [128,1024]-slot tag, bufs=4 (all 8
   banks). Phase 1 holds exactly 4 slots (k chunks x2, v x2 with eight
   [128,128] accumulator slices each; start=True only on each bank's
   first slice since it clears the whole bank's has_written bits), so
   k+v trail the x stream concurrently; attention cycles the same ring
   for score pairs, AV groups, transposes, and out-projection halves;
 - kT/v copies ride the ACT engine, the DVE handles only q casts;
 - o-groups trail their chunk's exps by >= 4 score-pairs, their PE
   transposes are deferred one slot behind the DVE normalization, and
   the out-projection is spread one half-tile per slot, so neither
   ACT, DVE, nor the PSUM ring gates the PE.
"""

import sys

sys.path.insert(0, "/opt/trn_rl_repo")

import numpy as np
import ml_dtypes

import concourse.bass as bass
import concourse.mybir as mybir
import concourse.tile as tile
from concourse import bacc
from concourse.bass_utils import run_bass_kernel_spmd
from concourse.masks import make_identity

BF16 = mybir.dt.bfloat16
F32 = mybir.dt.float32
bf16 = ml_dtypes.bfloat16

B, N, E = 2, 2048, 2048
H, D, G = 16, 128, 4
HKV = H // G
JL = G * D                     # 512 local q-head dims per core
ET = E // 128                  # 16
NT = N // 128                  # 16
CH = N // 512                  # 4
SCALE = 1.0 / float(np.sqrt(D))

_cached = {}


def _build(iters=1):
    nc = bacc.Bacc("TRN2", target_bir_lowering=False, debug=False, num_devices=8)

    xT = nc.dram_tensor("xT", [E, N], BF16, kind="ExternalInput")
    wq = nc.dram_tensor("wq", [E, JL], BF16, kind="ExternalInput")
    wk = nc.dram_tensor("wk", [E, D], BF16, kind="ExternalInput")
    wv = nc.dram_tensor("wv", [E, D], BF16, kind="ExternalInput")
    wo = nc.dram_tensor("wo", [JL, E], BF16, kind="ExternalInput")
    out = nc.dram_tensor("out", [N, E], BF16, kind="ExternalOutput")

    with tile.TileContext(nc) as tc:
        with (
            tc.tile_pool(name="const", bufs=1) as cpool,
            tc.tile_pool(name="xp", bufs=1) as xpool,
            tc.tile_pool(name="wp", bufs=1) as wpool,
            tc.tile_pool(name="kvp", bufs=1) as kvpool,
            tc.tile_pool(name="qp", bufs=1) as qpool,
            tc.tile_pool(name="pp", bufs=3) as ppool,
            tc.tile_pool(name="op", bufs=4) as opool,
            tc.tile_pool(name="otp", bufs=1) as otpool,
            tc.tile_pool(name="outp", bufs=4) as outpool,
            tc.tile_pool(name="ps1", bufs=4, space="PSUM") as P1,
        ):
            ident = cpool.tile([128, 128], BF16, tag="ident")
            make_identity(nc, ident[:])

            for _ in range(iters):
                _emit_iter(nc, tc, ident, xpool, wpool, kvpool, qpool, ppool,
                           opool, otpool, outpool, P1,
                           xT, wq, wk, wv, wo, out)

    nc.compile()
    return nc


def _emit_iter(nc, tc, ident, xpool, wpool, kvpool, qpool, ppool, opool,
               otpool, outpool, P1, xT, wq, wk, wv, wo, out):
    x_sb = xpool.tile([128, ET, N], BF16, tag="x")
    wq_sb = wpool.tile([128, ET, JL], BF16, tag="wq")
    wk_sb = wpool.tile([128, ET, D], BF16, tag="wk")
    wv_sb = wpool.tile([128, ET, D], BF16, tag="wv")
    wo_sb = wpool.tile([128, G, E], BF16, tag="wo")
    kT_sb = kvpool.tile([128, N], BF16, tag="kT")
    v_sb = kvpool.tile([128, NT, 130], BF16, tag="v")
    qT_sb = qpool.tile([128, G, N], BF16, tag="qT")
    oT_sb = otpool.tile([128, G, N], BF16, tag="oT")

    # --- input DMAs, in consumption order ---
    # Only sync and scalar drive fast hardware DMA queues (~140GB/s
    # each). The gpsimd queue is software-driven (~35GB/s) AND its slow
    # descriptors hog the shared DMA credit pool, freezing the fast
    # queues — so it gets nothing. Everything rides the two fast queues
    # in consumption order: x0/wk first, wq/wo behind x (needed late).
    xr = xT.rearrange("(a p) n -> p a n", p=128)
    wkr = wk.rearrange("(a p) d -> p a d", p=128)
    wvr = wv.rearrange("(a p) d -> p a d", p=128)
    wqr = wq.rearrange("(a p) j -> p a j", p=128)
    nc.scalar.dma_start(x_sb[:, 0, 0:1024], xr[:, 0, 0:1024])
    nc.sync.dma_start(wk_sb[:, 0:8, :], wkr[:, 0:8, :])
    nc.scalar.dma_start(x_sb[:, 0, 1024:2048], xr[:, 0, 1024:2048])
    nc.sync.dma_start(wk_sb[:, 8:16, :], wkr[:, 8:16, :])
    nc.scalar.dma_start(wv_sb[:, 0:8, :], wvr[:, 0:8, :])
    nc.scalar.dma_start(x_sb[:, 1, :], xr[:, 1, :])
    nc.sync.dma_start(wv_sb[:, 8:16, :], wvr[:, 8:16, :])
    # x15 rides the slow gpsimd queue: issued up front it finishes ~37us,
    # right when the k/v et-loop reaches it, and a single credit can't
    # clog the shared descriptor pool.
    nc.gpsimd.dma_start(x_sb[:, 15, :], xr[:, 15, :])
    for et in range(2, ET - 1):
        eng = nc.sync if et % 2 == 0 else nc.scalar
        eng.dma_start(x_sb[:, et, :], xr[:, et, :])
    for g in range(G):
        nc.sync.dma_start(wq_sb[:, :, g * 128:(g + 1) * 128],
                          wqr[:, :, g * 128:(g + 1) * 128])
    for jt in range(G):
        nc.scalar.dma_start(wo_sb[:, jt, :], wo[jt * 128:(jt + 1) * 128, :])

    nc.vector.memset(v_sb[:, :, 128:129], 1.0)

    # --- phase 1 (own PSUM scope: k 4 banks + v 4 banks) ---
    # kT: 4 chunk accumulators (2 double-bank tiles) and ALL 16 v s-tile
    # accumulators (4 per bank as [128,128] f32 slices) run concurrently,
    # so the whole k+v projection (28us of PE) trails the x DMA stream
    # (~30us) with no starvation window.
    if True:
        # PE warmup: dummy matmuls on the identity while the first DMAs
        # are in flight, ramping the PE out of its low p-state.
        for i in range(24):
            wmt = P1.tile([128, 512], F32, tag="mm", name=f"warm{i}")
            nc.tensor.matmul(wmt[:, 0:128], ident[:], ident[:],
                             start=True, stop=True)

        kp = [P1.tile([128, 1024], F32, tag="mm", name=f"kp{_i}")
              for _i in range(2)]
        kps = [kp[_i // 2][:, (_i % 2) * 512:(_i % 2 + 1) * 512]
               for _i in range(CH)]
        vacc = [P1.tile([128, 1024], F32, tag="mm", name=f"vacc{_i}")
                for _i in range(2)]
        vps = [vacc[_s // 8][:, (_s % 8) * 128:(_s % 8 + 1) * 128]
               for _s in range(NT)]
        # The v et-steps lag k's by 4, so early PE consumption (~0.85us/et
        # k-only) matches the slower early x arrivals, then k+v (~2.9us/et)
        # trails the steady stream with no starvation.
        def v_step(et):
            for st in range(NT):
                # start=True clears has_written for the WHOLE bank, so only
                # the first slice of each 4-slice bank may use it; the other
                # slices' first writes overwrite-and-set on cleared bits,
                # which is exactly accumulation-start semantics.
                nc.tensor.matmul(
                    vps[st], x_sb[:, et, st * 128:(st + 1) * 128],
                    wv_sb[:, et, :],
                    start=(et == 0 and st % 4 == 0), stop=(et == ET - 1),
                )

        for et in range(ET):
            for sc in range(CH):
                nc.tensor.matmul(
                    kps[sc], wk_sb[:, et, :], x_sb[:, et, sc * 512:(sc + 1) * 512],
                    start=(et == 0), stop=(et == ET - 1),
                )
            if et >= 4:
                v_step(et - 4)
        for et in range(ET - 4, ET):
            v_step(et)
        # kT/v copies go on the (otherwise idle) ACT engine so the DVE
        # queue holds nothing but the q-pair casts (their P1 ring readers).
        kv_copies = [lambda sc=sc: nc.scalar.copy(
                         kT_sb[:, sc * 512:(sc + 1) * 512], kps[sc])
                     for sc in range(CH)]
        kv_copies += [lambda st=st: nc.scalar.copy(v_sb[:, st, 0:128], vps[st])
                      for st in range(NT)]

        def emit_q_pair(q0, q1):
            ps = P1.tile([128, 1024], F32, tag="mm")
            for half, (g, ncg) in enumerate((q0, q1)):
                sl = ps[:, half * 512:(half + 1) * 512]
                for et in range(ET):
                    nc.tensor.matmul(
                        sl, wq_sb[:, et, g * 128:(g + 1) * 128],
                        x_sb[:, et, ncg * 512:(ncg + 1) * 512],
                        start=(et == 0), stop=(et == ET - 1),
                    )
                nc.vector.tensor_copy(qT_sb[:, g, ncg * 512:(ncg + 1) * 512], sl)

        # kp0's copies must precede qp0 (P1 ring), kp1's must precede qp1;
        # the v copies (needed only by attention) fill the remaining gaps.
        qlist = [(g, ncg) for g in range(G) for ncg in range(CH)]
        for i in range(8):
            if i < 2:
                kv_copies.pop(0)()
                kv_copies.pop(0)()
            emit_q_pair(qlist[2 * i], qlist[2 * i + 1])
            for _ in range(3):
                if kv_copies:
                    kv_copies.pop(0)()

    # --- phase 2 + 3, pipelined per chunk of 512 n-columns ---
    # Scores for two s-tiles share one double-bank psum tile so a single
    # (wider, cheaper per element) Exp covers both. o-groups trail their
    # chunk by >= 2 score-pairs so the PE never catches the ACT engine;
    # one out-projection n-tile is emitted per sub-slot once a column's
    # four heads are done.
    if True:
        def emit_score_pair(g, c, p_t, sp):
            ps = P1.tile([128, 1024], F32, tag="mm")
            for half in range(2):
                st = 2 * sp + half
                nc.tensor.matmul(
                    ps[:, half * 512:(half + 1) * 512],
                    kT_sb[:, st * 128:(st + 1) * 128],
                    qT_sb[:, g, c * 512:(c + 1) * 512],
                    start=True, stop=True,
                )
            nc.scalar.activation(
                p_t[:, 2 * sp * 512:(2 * sp + 2) * 512], ps[:],
                mybir.ActivationFunctionType.Exp, scale=SCALE,
            )

        def emit_o_group_av(g, c, p_t, t):
            """AV matmuls + normalization (DVE); returns the transpose step,
            which the caller defers a slot so the PE never waits on the DVE."""
            pso = P1.tile([128, 130], F32, tag="mm")
            for st in range(NT):
                nc.tensor.matmul(
                    pso[:, 0:129], p_t[:, st * 512 + t * 128: st * 512 + (t + 1) * 128],
                    v_sb[:, st, 0:129],
                    start=(st == 0), stop=(st == NT - 1),
                )
            rc = opool.tile([128, 1], F32, tag="recip")
            nc.vector.reciprocal(rc[:], pso[:, 128:129])
            o_n = opool.tile([128, 128], BF16, tag="o_n")
            nc.vector.tensor_scalar_mul(o_n[:], pso[:, 0:128], rc[:])
            # DMA-XBAR transpose straight into oT: off the PE entirely, and
            # one op replaces the PE transpose + DVE copy. Consumers of oT
            # run >= 3 slots later, hiding the DMA latency.
            nc.sync.dma_start_transpose(
                oT_sb[:, g, c * 512 + t * 128: c * 512 + (t + 1) * 128], o_n[:],
            )
            return None

        def emit_out_half(nt, half):
            ps = P1.tile([128, 1024], F32, tag="mm")
            pe2 = [ps[:, 0:512], ps[:, 512:1024]]
            for e2 in range(2):
                ec = half * 2 + e2
                for g in range(G):
                    nc.tensor.matmul(
                        pe2[e2], oT_sb[:, g, nt * 128:(nt + 1) * 128],
                        wo_sb[:, g, ec * 512:(ec + 1) * 512],
                        start=(g == 0), stop=(g == G - 1),
                    )
            stage = outpool.tile([128, 1024], BF16, tag="out")
            # In the tail column the second cast runs on the (then idle)
            # ACT engine, splitting the drain across both copy engines.
            nc.vector.tensor_copy(stage[:, 0:512], pe2[0])
            if nt >= 12:
                nc.scalar.copy(stage[:, 512:1024], pe2[1])
            else:
                nc.vector.tensor_copy(stage[:, 512:1024], pe2[1])
            nc.sync.dma_start(
                out[nt * 128:(nt + 1) * 128, half * 1024:(half + 1) * 1024],
                stage[:],
            )

        # pending o-group / out-half work queue: one entry per "slot"
        # (after each sub's score pairs), two when backed up. o-groups
        # never pop at a chunk's first slot (guarantees >= 4 score-pairs
        # of exp lead); their transposes are deferred one slot (returned
        # as follow-ups) so the PE never waits on the DVE normalization.
        chunks = [(c, g) for c in range(CH) for g in range(G)]
        pending = []
        follow = []

        def pop_one():
            kind, fn = pending.pop(0)
            r = fn()
            if r is not None:
                follow.append(r)

        def pop_slot(sub):
            while follow:
                follow.pop(0)()
            if pending and not (sub == 0 and pending[0][0] == "og"):
                pop_one()
            if len(pending) > 5:
                pop_one()

        for i, (c, g) in enumerate(chunks):
            p_t = ppool.tile([128, NT * 512], BF16, tag="p", name=f"p{i}")
            for sub in range(4):
                emit_score_pair(g, c, p_t, 2 * sub)
                emit_score_pair(g, c, p_t, 2 * sub + 1)
                pop_slot(sub)
            # queue this chunk's o-groups (consumed over the next chunks);
            # when the column's 4 heads are done, interleave the column's
            # out-projection halves behind the o-groups that produce them.
            og = [("og", lambda g=g, c=c, p_t=p_t, t=t: emit_o_group_av(g, c, p_t, t))
                  for t in range(4)]
            if g == G - 1:
                oh = [("oh", lambda nt=4 * c + t, h=h: emit_out_half(nt, h))
                      for t in range(4) for h in range(2)]
                pending.extend([og[0], og[1], og[2], oh[0], og[3], oh[1],
                                oh[2], oh[3], oh[4], oh[5], oh[6], oh[7]])
            else:
                pending.extend(og)
        while pending:
            pop_slot(1)
        while follow:
            follow.pop(0)()


def get_nc(iters=1):
    key = ("nc", iters)
    if key not in _cached:
        _cached[key] = _build(iters)
    return _cached[key]


def make_in_maps(x, Wq, Wk, Wv, Wo):
    """Per-core host-side sharding. Core c -> (b=c//4, gk=c%4)."""
    in_maps = []
    xT = [np.ascontiguousarray(x[b].T).astype(bf16) for b in range(B)]
    wq_s = [np.ascontiguousarray(Wq[gk * JL:(gk + 1) * JL, :].T).astype(bf16)
            for gk in range(HKV)]
    wk_s = [np.ascontiguousarray(Wk[gk * D:(gk + 1) * D, :].T).astype(bf16)
            for gk in range(HKV)]
    wv_s = [np.ascontiguousarray(Wv[gk * D:(gk + 1) * D, :].T).astype(bf16)
            for gk in range(HKV)]
    wo_s = [np.ascontiguousarray(Wo[:, gk * JL:(gk + 1) * JL].T).astype(bf16)
            for gk in range(HKV)]
    for c in range(8):
        b, gk = c // 4, c % 4
        in_maps.append({
            "xT": xT[b], "wq": wq_s[gk], "wk": wk_s[gk],
            "wv": wv_s[gk], "wo": wo_s[gk],
        })
    return in_maps


def kernel(x, Wq, Wk, Wv, Wo):
    nc = get_nc()
    in_maps = make_in_maps(x, Wq, Wk, Wv, Wo)
    res = run_bass_kernel_spmd(nc, in_maps, core_ids=list(range(8)))
    out = np.empty((B, N, E), np.float32)
    for b in range(B):
        acc = res.results[b * 4]["out"].astype(np.float32)
        for gk in range(1, HKV):
            acc = acc + res.results[b * 4 + gk]["out"].astype(np.float32)
        out[b] = acc
    return out


# revision 38
# speedup vs baseline: 1.2151x; 1.0041x over previous
"""GQA attention kernel for 8 Trainium2 NeuronCores.

Sharding: core c -> (b = c // 4, kv-group gk = c % 4).
Each core computes, for its batch b and its kv head gk (which owns the 4
contiguous q-heads gk*4..gk*4+3):
    q/k/v projections, attention, and a partial out-projection
    out_partial[b] = o_heads @ Wo[:, gk*512:(gk+1)*512].T
Host sums the 4 partials per batch (bf16 partials, f32 accumulate).

All matmuls in bf16 (fp32 PSUM accumulation). Softmax without max
subtraction (scores are bounded ~|4.5| at this problem's weight scale);
row sums come free from a ones-column appended to V; normalization is
applied to the 128-wide per-head output ahead of the out projection.

Layout (per core), everything E/K-major for the PE:
  xT  [E, N]   = x[b].T          kT [128d, N]    scoresT [s, n] chunks
  wq  [E, 512] = Wq rows.T       qT [128, 4g, N]
  wk  [E, 128] = Wk rows.T       v  [128, 16st, 130] (col 128 = ones)
  wv  [E, 128]                   oT [128, 4g, N]
  wo  [512, E] = Wo cols.T       out [N, E] bf16 partial

Schedule notes (from NTFF profiling):
 - only sync/scalar drive fast (~140GB/s) HW DMA queues; gpsimd's
   software queue is ~35GB/s and its slow descriptors hog the shared
   credit pool, so it carries just one late x chunk. All input DMAs are
   issued in consumption order so the first k matmul starts ~10us in;
 - dummy matmuls on the identity ramp the PE out of its low p-state
   while the first x chunk is in flight;
 - one unified PSUM ring: a single [128,1024]-slot tag, bufs=4 (all 8
   banks). Phase 1 holds exactly 4 slots (k chunks x2, v x2 with eight
   [128,128] accumulator slices each; start=True only on each bank's
   first slice since it clears the whole bank's has_written bits), so
   k+v trail the x stream concurrently; attention cycles the same ring
   for score pairs, AV groups, transposes, and out-projection halves;
 - kT/v copies ride the ACT engine, the DVE handles only q casts;
 - o-groups trail their chunk's exps by >= 4 score-pairs, the o
   transposes go through the DMA XBAR (sync queue) straight into oT —
   off the PE and DVE entirely — and the out-projection is spread one
   half-tile per slot, so no engine or ring gates the PE.
"""

import sys

sys.path.insert(0, "/opt/trn_rl_repo")

import numpy as np
import ml_dtypes

import concourse.bass as bass
import concourse.mybir as mybir
import concourse.tile as tile
from concourse import bacc
from concourse.bass_utils import run_bass_kernel_spmd
from concourse.masks import make_identity

BF16 = mybir.dt.bfloat16
F32 = mybir.dt.float32
bf16 = ml_dtypes.bfloat16

B, N, E = 2, 2048, 2048
H, D, G = 16, 128, 4
HKV = H // G
JL = G * D                     # 512 local q-head dims per core
ET = E // 128                  # 16
NT = N // 128                  # 16
CH = N // 512                  # 4
SCALE = 1.0 / float(np.sqrt(D))

_cached = {}


def _build(iters=1):
    nc = bacc.Bacc("TRN2", target_bir_lowering=False, debug=False, num_devices=8)

    xT = nc.dram_tensor("xT", [E, N], BF16, kind="ExternalInput")
    wq = nc.dram_tensor("wq", [E, JL], BF16, kind="ExternalInput")
    wk = nc.dram_tensor("wk", [E, D], BF16, kind="ExternalInput")
    wv = nc.dram_tensor("wv", [E, D], BF16, kind="ExternalInput")
    wo = nc.dram_tensor("wo", [JL, E], BF16, kind="ExternalInput")
    out = nc.dram_tensor("out", [N, E], BF16, kind="ExternalOutput")

    with tile.TileContext(nc) as tc:
        with (
            tc.tile_pool(name="const", bufs=1) as cpool,
            tc.tile_pool(name="xp", bufs=1) as xpool,
            tc.tile_pool(name="wp", bufs=1) as wpool,
            tc.tile_pool(name="kvp", bufs=1) as kvpool,
            tc.tile_pool(name="qp", bufs=1) as qpool,
            tc.tile_pool(name="pp", bufs=3) as ppool,
            tc.tile_pool(name="op", bufs=4) as opool,
            tc.tile_pool(name="otp", bufs=1) as otpool,
            tc.tile_pool(name="outp", bufs=4) as outpool,
            tc.tile_pool(name="ps1", bufs=4, space="PSUM") as P1,
        ):
            ident = cpool.tile([128, 128], BF16, tag="ident")
            make_identity(nc, ident[:])

            for _ in range(iters):
                _emit_iter(nc, tc, ident, xpool, wpool, kvpool, qpool, ppool,
                           opool, otpool, outpool, P1,
                           xT, wq, wk, wv, wo, out)

    nc.compile()
    return nc


def _emit_iter(nc, tc, ident, xpool, wpool, kvpool, qpool, ppool, opool,
               otpool, outpool, P1, xT, wq, wk, wv, wo, out):
    x_sb = xpool.tile([128, ET, N], BF16, tag="x")
    wq_sb = wpool.tile([128, ET, JL], BF16, tag="wq")
    wk_sb = wpool.tile([128, ET, D], BF16, tag="wk")
    wv_sb = wpool.tile([128, ET, D], BF16, tag="wv")
    wo_sb = wpool.tile([128, G, E], BF16, tag="wo")
    kT_sb = kvpool.tile([128, N], BF16, tag="kT")
    v_sb = kvpool.tile([128, NT, 130], BF16, tag="v")
    qT_sb = qpool.tile([128, G, N], BF16, tag="qT")
    oT_sb = otpool.tile([128, G, N], BF16, tag="oT")

    # --- input DMAs, in consumption order ---
    # Only sync and scalar drive fast hardware DMA queues (~140GB/s
    # each). The gpsimd queue is software-driven (~35GB/s) AND its slow
    # descriptors hog the shared DMA credit pool, freezing the fast
    # queues — so it gets nothing. Everything rides the two fast queues
    # in consumption order: x0/wk first, wq/wo behind x (needed late).
    xr = xT.rearrange("(a p) n -> p a n", p=128)
    wkr = wk.rearrange("(a p) d -> p a d", p=128)
    wvr = wv.rearrange("(a p) d -> p a d", p=128)
    wqr = wq.rearrange("(a p) j -> p a j", p=128)
    nc.scalar.dma_start(x_sb[:, 0, 0:1024], xr[:, 0, 0:1024])
    nc.sync.dma_start(wk_sb[:, 0:8, :], wkr[:, 0:8, :])
    nc.scalar.dma_start(x_sb[:, 0, 1024:2048], xr[:, 0, 1024:2048])
    # x1/x2 jump ahead of wv and wk's second half: with the v et-steps
    # lagging k by 4, wv isn't consumed until ~16us, but k-et1/et2 want
    # x1/x2 at ~11-13us.
    nc.scalar.dma_start(x_sb[:, 1, :], xr[:, 1, :])
    nc.sync.dma_start(wk_sb[:, 8:16, :], wkr[:, 8:16, :])
    nc.scalar.dma_start(wv_sb[:, 0:8, :], wvr[:, 0:8, :])
    nc.sync.dma_start(wv_sb[:, 8:16, :], wvr[:, 8:16, :])
    # x15 rides the slow gpsimd queue: issued up front it finishes ~37us,
    # right when the k/v et-loop reaches it, and a single credit can't
    # clog the shared descriptor pool.
    nc.gpsimd.dma_start(x_sb[:, 15, :], xr[:, 15, :])
    for et in range(2, ET - 1):
        eng = nc.sync if et % 2 == 0 else nc.scalar
        eng.dma_start(x_sb[:, et, :], xr[:, et, :])
    for g in range(G):
        nc.sync.dma_start(wq_sb[:, :, g * 128:(g + 1) * 128],
                          wqr[:, :, g * 128:(g + 1) * 128])
    for jt in range(G):
        nc.scalar.dma_start(wo_sb[:, jt, :], wo[jt * 128:(jt + 1) * 128, :])

    nc.vector.memset(v_sb[:, :, 128:129], 1.0)

    # --- phase 1 (own PSUM scope: k 4 banks + v 4 banks) ---
    # kT: 4 chunk accumulators (2 double-bank tiles) and ALL 16 v s-tile
    # accumulators (4 per bank as [128,128] f32 slices) run concurrently,
    # so the whole k+v projection (28us of PE) trails the x DMA stream
    # (~30us) with no starvation window.
    if True:
        # PE warmup: dummy matmuls on the identity while the first DMAs
        # are in flight, ramping the PE out of its low p-state.
        for i in range(24):
            wmt = P1.tile([128, 512], F32, tag="mm", name=f"warm{i}")
            nc.tensor.matmul(wmt[:, 0:128], ident[:], ident[:],
                             start=True, stop=True)

        kp = [P1.tile([128, 1024], F32, tag="mm", name=f"kp{_i}")
              for _i in range(2)]
        kps = [kp[_i // 2][:, (_i % 2) * 512:(_i % 2 + 1) * 512]
               for _i in range(CH)]
        vacc = [P1.tile([128, 1024], F32, tag="mm", name=f"vacc{_i}")
                for _i in range(2)]
        vps = [vacc[_s // 8][:, (_s % 8) * 128:(_s % 8 + 1) * 128]
               for _s in range(NT)]
        # The v et-steps lag k's by 4, so early PE consumption (~0.85us/et
        # k-only) matches the slower early x arrivals, then k+v (~2.9us/et)
        # trails the steady stream with no starvation.
        def v_step(et):
            for st in range(NT):
                # start=True clears has_written for the WHOLE bank, so only
                # the first slice of each 4-slice bank may use it; the other
                # slices' first writes overwrite-and-set on cleared bits,
                # which is exactly accumulation-start semantics.
                nc.tensor.matmul(
                    vps[st], x_sb[:, et, st * 128:(st + 1) * 128],
                    wv_sb[:, et, :],
                    start=(et == 0 and st % 4 == 0), stop=(et == ET - 1),
                )

        for et in range(ET):
            for sc in range(CH):
                nc.tensor.matmul(
                    kps[sc], wk_sb[:, et, :], x_sb[:, et, sc * 512:(sc + 1) * 512],
                    start=(et == 0), stop=(et == ET - 1),
                )
            if et >= 4:
                v_step(et - 4)
        for et in range(ET - 4, ET):
            v_step(et)
        # kT/v copies go on the (otherwise idle) ACT engine so the DVE
        # queue holds nothing but the q-pair casts (their P1 ring readers).
        kv_copies = [lambda sc=sc: nc.scalar.copy(
                         kT_sb[:, sc * 512:(sc + 1) * 512], kps[sc])
                     for sc in range(CH)]
        kv_copies += [lambda st=st: nc.scalar.copy(v_sb[:, st, 0:128], vps[st])
                      for st in range(NT)]

        def emit_q_pair(q0, q1):
            ps = P1.tile([128, 1024], F32, tag="mm")
            for half, (g, ncg) in enumerate((q0, q1)):
                sl = ps[:, half * 512:(half + 1) * 512]
                for et in range(ET):
                    nc.tensor.matmul(
                        sl, wq_sb[:, et, g * 128:(g + 1) * 128],
                        x_sb[:, et, ncg * 512:(ncg + 1) * 512],
                        start=(et == 0), stop=(et == ET - 1),
                    )
                nc.vector.tensor_copy(qT_sb[:, g, ncg * 512:(ncg + 1) * 512], sl)

        # kp0's copies must precede qp0 (P1 ring), kp1's must precede qp1;
        # the v copies (needed only by attention) fill the remaining gaps.
        qlist = [(g, ncg) for g in range(G) for ncg in range(CH)]
        for i in range(8):
            if i < 2:
                kv_copies.pop(0)()
                kv_copies.pop(0)()
            emit_q_pair(qlist[2 * i], qlist[2 * i + 1])
            for _ in range(3):
                if kv_copies:
                    kv_copies.pop(0)()

    # --- phase 2 + 3, pipelined per chunk of 512 n-columns ---
    # Scores for two s-tiles share one double-bank psum tile so a single
    # (wider, cheaper per element) Exp covers both. o-groups trail their
    # chunk by >= 2 score-pairs so the PE never catches the ACT engine;
    # one out-projection n-tile is emitted per sub-slot once a column's
    # four heads are done.
    if True:
        def emit_score_pair(g, c, p_t, sp):
            ps = P1.tile([128, 1024], F32, tag="mm")
            for half in range(2):
                st = 2 * sp + half
                nc.tensor.matmul(
                    ps[:, half * 512:(half + 1) * 512],
                    kT_sb[:, st * 128:(st + 1) * 128],
                    qT_sb[:, g, c * 512:(c + 1) * 512],
                    start=True, stop=True,
                )
            nc.scalar.activation(
                p_t[:, 2 * sp * 512:(2 * sp + 2) * 512], ps[:],
                mybir.ActivationFunctionType.Exp, scale=SCALE,
            )

        def emit_o_group_av(g, c, p_t, t):
            """AV matmuls + normalization (DVE); returns the transpose step,
            which the caller defers a slot so the PE never waits on the DVE."""
            pso = P1.tile([128, 130], F32, tag="mm")
            for st in range(NT):
                nc.tensor.matmul(
                    pso[:, 0:129], p_t[:, st * 512 + t * 128: st * 512 + (t + 1) * 128],
                    v_sb[:, st, 0:129],
                    start=(st == 0), stop=(st == NT - 1),
                )
            rc = opool.tile([128, 1], F32, tag="recip")
            nc.vector.reciprocal(rc[:], pso[:, 128:129])
            o_n = opool.tile([128, 128], BF16, tag="o_n")
            nc.vector.tensor_scalar_mul(o_n[:], pso[:, 0:128], rc[:])
            # DMA-XBAR transpose straight into oT: off the PE entirely, and
            # one op replaces the PE transpose + DVE copy. Consumers of oT
            # run >= 3 slots later, hiding the DMA latency.
            nc.sync.dma_start_transpose(
                oT_sb[:, g, c * 512 + t * 128: c * 512 + (t + 1) * 128], o_n[:],
            )
            return None

        def emit_out_half(nt, half):
            ps = P1.tile([128, 1024], F32, tag="mm")
            pe2 = [ps[:, 0:512], ps[:, 512:1024]]
            for e2 in range(2):
                ec = half * 2 + e2
                for g in range(G):
                    nc.tensor.matmul(
                        pe2[e2], oT_sb[:, g, nt * 128:(nt + 1) * 128],
                        wo_sb[:, g, ec * 512:(ec + 1) * 512],
                        start=(g == 0), stop=(g == G - 1),
                    )
            stage = outpool.tile([128, 1024], BF16, tag="out")
            # In the tail column the second cast runs on the (then idle)
            # ACT engine, splitting the drain across both copy engines.
            nc.vector.tensor_copy(stage[:, 0:512], pe2[0])
            if nt >= 12:
                nc.scalar.copy(stage[:, 512:1024], pe2[1])
            else:
                nc.vector.tensor_copy(stage[:, 512:1024], pe2[1])
            nc.sync.dma_start(
                out[nt * 128:(nt + 1) * 128, half * 1024:(half + 1) * 1024],
                stage[:],
            )

        # pending o-group / out-half work queue: one entry per "slot"
        # (after each sub's score pairs), two when backed up. o-groups
        # never pop at a chunk's first slot (guarantees >= 4 score-pairs
        # of exp lead); their transposes are deferred one slot (returned
        # as follow-ups) so the PE never waits on the DVE normalization.
        chunks = [(c, g) for c in range(CH) for g in range(G)]
        pending = []
        follow = []

        def pop_one():
            kind, fn = pending.pop(0)
            r = fn()
            if r is not None:
                follow.append(r)

        def pop_slot(sub):
            while follow:
                follow.pop(0)()
            if pending and not (sub == 0 and pending[0][0] == "og"):
                pop_one()
            if len(pending) > 5:
                pop_one()

        for i, (c, g) in enumerate(chunks):
            p_t = ppool.tile([128, NT * 512], BF16, tag="p", name=f"p{i}")
            for sub in range(4):
                emit_score_pair(g, c, p_t, 2 * sub)
                emit_score_pair(g, c, p_t, 2 * sub + 1)
                pop_slot(sub)
            # queue this chunk's o-groups (consumed over the next chunks);
            # when the column's 4 heads are done, interleave the column's
            # out-projection halves behind the o-groups that produce them.
            og = [("og", lambda g=g, c=c, p_t=p_t, t=t: emit_o_group_av(g, c, p_t, t))
                  for t in range(4)]
            if g == G - 1:
                oh = [("oh", lambda nt=4 * c + t, h=h: emit_out_half(nt, h))
                      for t in range(4) for h in range(2)]
                pending.extend([og[0], og[1], og[2], oh[0], og[3], oh[1],
                                oh[2], oh[3], oh[4], oh[5], oh[6], oh[7]])
            else:
                pending.extend(og)
        while pending:
            pop_slot(1)
        while follow:
            follow.pop(0)()


def get_nc(iters=1):
    key = ("nc", iters)
    if key not in _cached:
        _cached[key] = _build(iters)
    return _cached[key]


def make_in_maps(x, Wq, Wk, Wv, Wo):
    """Per-core host-side sharding. Core c -> (b=c//4, gk=c%4)."""
    in_maps = []
    xT = [np.ascontiguousarray(x[b].T).astype(bf16) for b in range(B)]
    wq_s = [np.ascontiguousarray(Wq[gk * JL:(gk + 1) * JL, :].T).astype(bf16)
            for gk in range(HKV)]
    wk_s = [np.ascontiguousarray(Wk[gk * D:(gk + 1) * D, :].T).astype(bf16)
            for gk in range(HKV)]
    wv_s = [np.ascontiguousarray(Wv[gk * D:(gk + 1) * D, :].T).astype(bf16)
            for gk in range(HKV)]
    wo_s = [np.ascontiguousarray(Wo[:, gk * JL:(gk + 1) * JL].T).astype(bf16)
            for gk in range(HKV)]
    for c in range(8):
        b, gk = c // 4, c % 4
        in_maps.append({
            "xT": xT[b], "wq": wq_s[gk], "wk": wk_s[gk],
            "wv": wv_s[gk], "wo": wo_s[gk],
        })
    return in_maps


def kernel(x, Wq, Wk, Wv, Wo):
    nc = get_nc()
    in_maps = make_in_maps(x, Wq, Wk, Wv, Wo)
    res = run_bass_kernel_spmd(nc, in_maps, core_ids=list(range(8)))
    out = np.empty((B, N, E), np.float32)
    for b in range(B):
        acc = res.results[b * 4]["out"].astype(np.float32)
        for gk in range(1, HKV):
            acc = acc + res.results[b * 4 + gk]["out"].astype(np.float32)
        out[b] = acc
    return out
